# revision 28
# baseline (speedup 1.0000x reference)
"""Multi-head causal self-attention on 8 Trainium2 NeuronCores.

Sharding: core = (batch b, head-half). Each of the 8 cores computes
attention for 8 of the 16 heads of one of the 4 batch elements, plus the
partial output projection over its 512 feature columns. Host sums the two
partial projections per batch and adds the bias.

All device tensors are kept transposed (feature-major) so every matmul
contraction lands on the partition axis:
  QK^T:  S^T[k,q] = K^T_blk.T @ Q^T_chunk           (contraction d=64)
  AV:    outT[d,q] = V_ext_blk.T @ expS^T_blk       (contraction k=128)
V carries an extra ones-column so row 64 of the AV accumulator is the
softmax denominator. Causal masking of the diagonal-band tiles is done
inside the S^T accumulation group with a (-1e30 * I) @ band_mask matmul:
for the tile at k-block offset rb within a q-chunk, columns q < 128*rb + k
get -1e30 added before the exp.
"""

import numpy as np

import concourse.bass as bass
import concourse.tile as tile
from concourse import bacc, mybir
from concourse import bass_utils

F32 = mybir.dt.float32
AF = mybir.ActivationFunctionType

B, T, D, H, HD = 4, 2048, 1024, 16, 64
N_CORES = 8
HL = 8          # heads per core (local)
CB = 8          # c (contraction) blocks of 128
TB = 16         # t blocks of 128
TC = 4          # t chunks of 512
GS = 2          # k-blocks per exp group (staging = [128, 512*GS] psum)
NEG = -1.0e30
HEADS_EMIT = HL  # reduced during debugging bisection

_CACHED_NC = None


def _emit(tc, xT, wqkT, wvT, wpT, stepm, negI, projT, phase="all"):
    nc = tc.nc
    from contextlib import ExitStack

    with ExitStack() as ctx:
        # ---- pools (phase-scoped so SBUF space is reused) ----
        consts = ctx.enter_context(tc.tile_pool(name="consts", bufs=1))
        psum = ctx.enter_context(tc.tile_pool(name="psum", bufs=1, space="PSUM"))

        vtp = ctx.enter_context(tc.tile_pool(name="vtp", bufs=1))
        qkp = ctx.enter_context(tc.tile_pool(name="qkp", bufs=1))

        es_qkv = ctx.enter_context(ExitStack())   # x, wv, wq streams: qkv only
        xsp = es_qkv.enter_context(tc.tile_pool(name="xsp", bufs=1))
        wvp = es_qkv.enter_context(tc.tile_pool(name="wvp", bufs=1))
        wqsp = es_qkv.enter_context(tc.tile_pool(name="wqsp", bufs=16))

        # ---- consts ----
        bm_t = consts.tile([128, 512], F32, name="bm_t")
        nc.sync.dma_start(out=bm_t, in_=stepm)
        negI_t = consts.tile([128, 128], F32, name="negI_t")
        nc.sync.dma_start(out=negI_t, in_=negI)

        # ---- load x^T (and V weights) ----
        xs = []
        for cb in range(CB):
            x_t = xsp.tile([128, T], F32, name=f"xs{cb}")
            nc.sync.dma_start(out=x_t, in_=xT[cb * 128:(cb + 1) * 128, :])
            xs.append(x_t)
        wv = []
        for cb in range(CB):
            wv_t = wvp.tile([128, 512], F32, name=f"wv{cb}")
            nc.sync.dma_start(out=wv_t, in_=wvT[cb * 128:(cb + 1) * 128, :])
            wv.append(wv_t)

        # ---- V = x @ Wv^T, stored [128, 8 heads, 65] with ones col ----
        vt = []
        for tb in range(TB):
            ps = psum.tile([128, 512], F32, tag="acc", bufs=2, name=f"vps{tb}")
            for cb in range(CB):
                nc.tensor.matmul(
                    ps, lhsT=xs[cb][:, tb * 128:(tb + 1) * 128], rhs=wv[cb],
                    start=(cb == 0), stop=(cb == CB - 1))
            v_t = vtp.tile([128, HL, 65], F32, name=f"vt{tb}")
            nc.vector.memset(v_t[:, :, 64:65], 1.0)
            nc.vector.tensor_copy(
                out=v_t[:, :, 0:64],
                in_=ps.rearrange("p (h d) -> p h d", h=HL))
            vt.append(v_t)

        # ---- QK^T features: qk[fb][128, T]; fb 0-3 = Q, 4-7 = K ----
        qk = [None] * 8
        for fb in range(8):
            wqs = []
            for cb in range(CB):
                w_t = wqsp.tile([128, 128], F32, tag="wqs", name=f"wq{fb}_{cb}")
                nc.sync.dma_start(out=w_t, in_=wqkT[fb, cb])
                wqs.append(w_t)
            qk_t = qkp.tile([128, T], F32, name=f"qk{fb}")
            for tcc in range(TC):
                ps = psum.tile([128, 512], F32, tag="acc", bufs=2,
                               name=f"qkps{fb}_{tcc}")
                for cb in range(CB):
                    nc.tensor.matmul(
                        ps, lhsT=wqs[cb], rhs=xs[cb][:, tcc * 512:(tcc + 1) * 512],
                        start=(cb == 0), stop=(cb == CB - 1))
                nc.vector.tensor_copy(out=qk_t[:, tcc * 512:(tcc + 1) * 512], in_=ps)
            qk[fb] = qk_t

        if phase == "qkv":
            for fb in range(8):
                nc.sync.dma_start(out=projT[fb * 128:(fb + 1) * 128, :], in_=qk[fb])
            return

        es_qkv.close()  # free x / wv / wq stream space

        # ---- attention pools ----
        outup = ctx.enter_context(tc.tile_pool(name="outup", bufs=1))
        es_attn = ctx.enter_context(ExitStack())  # temp pools: attention only
        expp = es_attn.enter_context(tc.tile_pool(name="expp", bufs=3))
        rsp = es_attn.enter_context(tc.tile_pool(name="rsp", bufs=4))
        rallp = es_attn.enter_context(tc.tile_pool(name="rallp", bufs=2))
        drp = es_attn.enter_context(tc.tile_pool(name="drp", bufs=4, space="DRAM"))
        outU = [outup.tile([128, T], F32, name=f"outU{j}") for j in range(4)]
        if phase == "attn1":
            for j in range(4):
                nc.vector.memset(outU[j], 0.0)
        ral = None
        for h in range(HEADS_EMIT):
            hp, sub = h // 2, h % 2
            pb = sub * 64
            qT = qk[hp]
            kT = qk[4 + hp]
            if sub == 0:
                ral = rallp.tile([128, T], F32, tag="rall", name=f"ral{hp}")
            for qc in range(TC):
                nk = 4 * qc + 4
                ot = psum.tile([128, 512], F32, tag="ott", bufs=2, name=f"ot{h}_{qc}")
                for g in range(nk // GS):
                    st = psum.tile([128, 512 * GS], F32, tag="stag", bufs=2,
                                   name=f"st{h}_{qc}_{g}")
                    for kk in range(GS):
                        kb = g * GS + kk
                        rb = kb - 4 * qc
                        c0 = kk * 512
                        if rb < 0:
                            nc.tensor.matmul(
                                st[:, c0:c0 + 512],
                                lhsT=kT[pb:pb + 64, kb * 128:(kb + 1) * 128],
                                rhs=qT[pb:pb + 64, qc * 512:(qc + 1) * 512],
                                start=True, stop=True)
                        else:
                            w = 128 * rb + 128   # masked prefix width
                            lo = 384 - 128 * rb  # band-mask column offset
                            nc.tensor.matmul(
                                st[:, c0:c0 + 512],
                                lhsT=kT[pb:pb + 64, kb * 128:(kb + 1) * 128],
                                rhs=qT[pb:pb + 64, qc * 512:(qc + 1) * 512],
                                start=True, stop=False, skip_group_check=True)
                            # single K=128 mask matmul — two 64-row-tiled ones
                            # would run as concurrent row tiles on the same
                            # PSUM bank, which is fatal on HW
                            nc.tensor.matmul(
                                st[:, c0:c0 + w],
                                lhsT=negI_t, rhs=bm_t[:, lo:lo + w],
                                start=False, stop=True, skip_group_check=True)
                    ex = expp.tile([128, 512 * GS], F32, tag="expst",
                                   name=f"ex{h}_{qc}_{g}")
                    nc.scalar.activation(out=ex, in_=st, func=AF.Exp)
                    if phase == "attn1":
                        r0 = sub * 64
                        nc.vector.tensor_add(
                            outU[hp][r0:r0 + 64, qc * 512:(qc + 1) * 512],
                            outU[hp][r0:r0 + 64, qc * 512:(qc + 1) * 512],
                            ex[r0:r0 + 64, 0:512])
                        continue
                    for kk in range(GS):
                        kb = g * GS + kk
                        rb = kb - 4 * qc
                        off = 128 * rb if rb > 0 else 0
                        nc.tensor.matmul(
                            ot[0:65, off:512],
                            lhsT=vt[kb][:, h, :],
                            rhs=ex[:, kk * 512 + off:kk * 512 + 512],
                            start=(kb == 0), stop=(kb == nk - 1),
                            skip_group_check=True)
                if phase == "attn1":
                    continue
                # drain this q-chunk
                r0 = sub * 64
                nc.vector.tensor_copy(
                    out=outU[hp][r0:r0 + 64, qc * 512:(qc + 1) * 512],
                    in_=ot[0:64, :])
                if phase == "attn2":
                    continue
                rs = rsp.tile([128, 512], F32, tag="rs", name=f"rs{h}_{qc}")
                nc.vector.reciprocal(out=rs[64:65, :], in_=ot[64:65, :])
                # broadcast the reciprocal row across 64 partitions via a
                # DRAM bounce (DMA can replicate from a DRAM source)
                dr = drp.tile([1, 512], F32, tag="dr", name=f"dr{h}_{qc}")
                nc.sync.dma_start(out=dr, in_=rs[64:65, :])
                bc = bass.AP(tensor=dr.tensor, offset=dr.offset,
                             ap=[[0, 64]] + [list(d) for d in dr.ap])
                nc.sync.dma_start(
                    out=ral[r0:r0 + 64, qc * 512:(qc + 1) * 512], in_=bc)
            if sub == 1 and phase not in ("attn1", "attn2"):
                nc.vector.tensor_mul(outU[hp], outU[hp], ral)

        if phase in ("attn", "attn1", "attn2"):
            for j in range(4):
                nc.sync.dma_start(out=projT[j * 128:(j + 1) * 128, :], in_=outU[j])
            return

        # ---- partial projection: projT[o, t] = wpT.T @ outU ----
        es_attn.close()  # free attention temp space
        poutp = ctx.enter_context(tc.tile_pool(name="poutp", bufs=2))
        with tc.tile_pool(name="wpp", bufs=1) as wpp:
            wp = []
            for j in range(4):
                wp_t = wpp.tile([128, 1024], F32, name=f"wp{j}")
                nc.sync.dma_start(out=wp_t, in_=wpT[j * 128:(j + 1) * 128, :])
                wp.append(wp_t)
            for ob in range(8):
                po = poutp.tile([128, T], F32, tag="pout", name=f"po{ob}")
                for tcc in range(TC):
                    ps = psum.tile([128, 512], F32, tag="acc", bufs=2, name=f"pps{ob}_{tcc}")
                    for j in range(4):
                        nc.tensor.matmul(
                            ps, lhsT=wp[j][:, ob * 128:(ob + 1) * 128],
                            rhs=outU[j][:, tcc * 512:(tcc + 1) * 512],
                            start=(j == 0), stop=(j == 3))
                    nc.vector.tensor_copy(out=po[:, tcc * 512:(tcc + 1) * 512], in_=ps)
                nc.sync.dma_start(out=projT[ob * 128:(ob + 1) * 128, :], in_=po)


def build_nc(phase="all"):
    global _CACHED_NC
    if _CACHED_NC is not None and _CACHED_NC[0] == phase:
        return _CACHED_NC[1]
    nc = bacc.Bacc("TRN2", target_bir_lowering=False, debug=False,
                   num_devices=N_CORES)
    xT = nc.dram_tensor("xT", [D, T], F32, kind="ExternalInput").ap()
    wqkT = nc.dram_tensor("wqkT", [8, CB, 128, 128], F32, kind="ExternalInput").ap()
    wvT = nc.dram_tensor("wvT", [D, 512], F32, kind="ExternalInput").ap()
    wpT = nc.dram_tensor("wpT", [512, D], F32, kind="ExternalInput").ap()
    stepm = nc.dram_tensor("stepm", [128, 512], F32, kind="ExternalInput").ap()
    negI = nc.dram_tensor("negI", [128, 128], F32, kind="ExternalInput").ap()
    projT = nc.dram_tensor("projT", [D, T], F32, kind="ExternalOutput").ap()

    with tile.TileContext(nc) as t:
        _emit(t, xT, wqkT, wvT, wpT, stepm, negI, projT, phase=phase)
    nc.compile()
    _CACHED_NC = (phase, nc)
    return nc


def make_in_maps(x, W_qkv, W_proj):
    x = np.asarray(x, dtype=np.float32)
    W_qkv = np.asarray(W_qkv, dtype=np.float32)
    W_proj = np.asarray(W_proj, dtype=np.float32)

    # band mask: bm[k, c] = 1 iff c < k + 384  (see _emit diagonal masking)
    stepm = (np.arange(512)[None, :] <
             np.arange(128)[:, None] + 384).astype(np.float32)
    negI = (NEG * np.eye(128)).astype(np.float32)

    in_maps = []
    for core in range(N_CORES):
        b, half = core // 2, core % 2
        s = 512 * half
        # fold the 1/sqrt(HD) attention scale into the Q weights
        wq = W_qkv[s:s + 512] * np.float32(1.0 / np.sqrt(HD))
        wk = W_qkv[1024 + s:1024 + s + 512]
        wv = W_qkv[2048 + s:2048 + s + 512]
        wcatT = np.ascontiguousarray(np.concatenate([wq, wk], axis=0).T)  # [c, f]
        wqkT = np.ascontiguousarray(
            wcatT.reshape(8, 128, 8, 128).transpose(2, 0, 1, 3))  # [fb, cb, c, f]
        in_maps.append({
            "xT": np.ascontiguousarray(x[b].T),
            "wqkT": wqkT,
            "wvT": np.ascontiguousarray(wv.T),
            "wpT": np.ascontiguousarray(W_proj[:, s:s + 512].T),
            "stepm": stepm,
            "negI": negI,
        })
    return in_maps


def gather_output(results, b_proj):
    b_proj = np.asarray(b_proj, dtype=np.float32)
    out = np.empty((B, T, D), dtype=np.float32)
    for b in range(B):
        p = results[2 * b]["projT"] + results[2 * b + 1]["projT"]  # [D, T]
        out[b] = p.T + b_proj[None, :]
    return out


def run(x, W_qkv, W_proj, b_proj, trace=False, tmpdir=None):
    nc = build_nc()
    in_maps = make_in_maps(x, W_qkv, W_proj)
    if trace:
        bass_utils.upload_artifacts = lambda d: d
    res = bass_utils.run_bass_kernel_spmd(
        nc, in_maps, core_ids=list(range(N_CORES)), trace=trace, tmpdir=tmpdir)
    return gather_output(res.results, b_proj), res


def kernel(x, W_qkv, W_proj, b_proj):
    out, _ = run(x, W_qkv, W_proj, b_proj)
    return out


# revision 30
# speedup vs baseline: 2.4337x; 2.4337x over previous
"""Multi-head causal self-attention on 8 Trainium2 NeuronCores.

Sharding: core = (batch b, head-half). Each of the 8 cores computes
attention for 8 of the 16 heads of one of the 4 batch elements, plus the
partial output projection over its 512 feature columns. Host sums the two
partial projections per batch and adds the bias.

All device tensors are kept transposed (feature-major) so every matmul
contraction lands on the partition axis:
  QK^T:  S^T[k,q] = K^T_blk.T @ Q^T_chunk           (contraction d=64)
  AV:    outT[d,q] = V_ext_blk.T @ expS^T_blk       (contraction k=128)
V carries an extra ones-column so row 64 of the AV accumulator is the
softmax denominator. Causal masking of the diagonal-band tiles is done
inside the S^T accumulation group with a (-60000 * I) @ band_mask matmul.

Matmul operands are fp16 (fp32 streams at half rate on the TRN2 PE; fp16
products are exact in the fp32 accumulator, so the only loss is the input
rounding). Softmax statistics (row sums, reciprocals) stay fp32.
"""

import numpy as np

import concourse.bass as bass
import concourse.tile as tile
from concourse import bacc, mybir
from concourse import bass_utils

F32 = mybir.dt.float32
F16 = mybir.dt.float16
AF = mybir.ActivationFunctionType

B, T, D, H, HD = 4, 2048, 1024, 16, 64
N_CORES = 8
HL = 8          # heads per core (local)
CB = 8          # c (contraction) blocks of 128
TB = 16         # t blocks of 128
TC = 4          # t chunks of 512
GS = 2          # k-blocks per exp group (staging = [128, 512*GS] psum)
NEG = -60000.0  # fp16-safe mask value
HEADS_EMIT = HL

_CACHED_NC = None


def _emit(tc, xT, wqkT, wvT, wpT, stepm, negI, projT, phase="all"):
    nc = tc.nc
    from contextlib import ExitStack

    with ExitStack() as ctx:
        # ---- pools (phase-scoped so SBUF space is reused) ----
        consts = ctx.enter_context(tc.tile_pool(name="consts", bufs=1))
        psum = ctx.enter_context(tc.tile_pool(name="psum", bufs=1, space="PSUM"))

        vtp = ctx.enter_context(tc.tile_pool(name="vtp", bufs=1))
        qkp = ctx.enter_context(tc.tile_pool(name="qkp", bufs=1))

        es_qkv = ctx.enter_context(ExitStack())   # x, wv, wq streams: qkv only
        xsp = es_qkv.enter_context(tc.tile_pool(name="xsp", bufs=1))
        wvp = es_qkv.enter_context(tc.tile_pool(name="wvp", bufs=1))
        wqsp = es_qkv.enter_context(tc.tile_pool(name="wqsp", bufs=16))

        # ---- consts ----
        bm_t = consts.tile([128, 512], F16, name="bm_t")
        nc.sync.dma_start(out=bm_t, in_=stepm)
        negI_t = consts.tile([128, 128], F16, name="negI_t")
        nc.sync.dma_start(out=negI_t, in_=negI)

        # ---- load x^T (and V weights) ----
        xs = []
        for cb in range(CB):
            x_t = xsp.tile([128, T], F16, name=f"xs{cb}")
            nc.sync.dma_start(out=x_t, in_=xT[cb * 128:(cb + 1) * 128, :])
            xs.append(x_t)
        wv = []
        for cb in range(CB):
            wv_t = wvp.tile([128, 512], F16, name=f"wv{cb}")
            nc.sync.dma_start(out=wv_t, in_=wvT[cb * 128:(cb + 1) * 128, :])
            wv.append(wv_t)

        # ---- V = x @ Wv^T, stored [128, 8 heads, 65] with ones col ----
        vt = []
        for tb in range(TB):
            ps = psum.tile([128, 512], F32, tag="acc", bufs=2, name=f"vps{tb}")
            for cb in range(CB):
                nc.tensor.matmul(
                    ps, lhsT=xs[cb][:, tb * 128:(tb + 1) * 128], rhs=wv[cb],
                    start=(cb == 0), stop=(cb == CB - 1))
            v_t = vtp.tile([128, HL, 65], F16, name=f"vt{tb}")
            nc.vector.memset(v_t[:, :, 64:65], 1.0)
            nc.vector.tensor_copy(
                out=v_t[:, :, 0:64],
                in_=ps.rearrange("p (h d) -> p h d", h=HL))
            vt.append(v_t)

        # ---- QK^T features: qk[fb][128, T]; fb 0-3 = Q, 4-7 = K ----
        qk = [None] * 8
        for fb in range(8):
            wqs = []
            for cb in range(CB):
                w_t = wqsp.tile([128, 128], F16, tag="wqs", name=f"wq{fb}_{cb}")
                nc.sync.dma_start(out=w_t, in_=wqkT[fb, cb])
                wqs.append(w_t)
            qk_t = qkp.tile([128, T], F16, name=f"qk{fb}")
            for tcc in range(TC):
                ps = psum.tile([128, 512], F32, tag="acc", bufs=2,
                               name=f"qkps{fb}_{tcc}")
                for cb in range(CB):
                    nc.tensor.matmul(
                        ps, lhsT=wqs[cb], rhs=xs[cb][:, tcc * 512:(tcc + 1) * 512],
                        start=(cb == 0), stop=(cb == CB - 1))
                nc.vector.tensor_copy(out=qk_t[:, tcc * 512:(tcc + 1) * 512], in_=ps)
            qk[fb] = qk_t



        es_qkv.close()  # free x / wv / wq stream space

        # ---- attention pools ----
        outup = ctx.enter_context(tc.tile_pool(name="outup", bufs=1))
        es_attn = ctx.enter_context(ExitStack())  # temp pools: attention only
        expp = es_attn.enter_context(tc.tile_pool(name="expp", bufs=3))
        rsp = es_attn.enter_context(tc.tile_pool(name="rsp", bufs=4))
        rsegp = es_attn.enter_context(tc.tile_pool(name="rsegp", bufs=4))
        drp = es_attn.enter_context(tc.tile_pool(name="drp", bufs=4, space="DRAM"))
        outU = [outup.tile([128, T], F16, name=f"outU{j}") for j in range(4)]
        for h in range(HEADS_EMIT):
            hp, sub = h // 2, h % 2
            pb = sub * 64
            qT = qk[hp]
            kT = qk[4 + hp]
            for qc in range(TC):
                nk = 4 * qc + 4
                ot = psum.tile([128, 512], F32, tag="ott", bufs=2, name=f"ot{h}_{qc}")
                for g in range(nk // GS):
                    st = psum.tile([128, 512 * GS], F32, tag="stag", bufs=2,
                                   name=f"st{h}_{qc}_{g}")
                    for kk in range(GS):
                        kb = g * GS + kk
                        rb = kb - 4 * qc
                        c0 = kk * 512
                        if rb < 0:
                            nc.tensor.matmul(
                                st[:, c0:c0 + 512],
                                lhsT=kT[pb:pb + 64, kb * 128:(kb + 1) * 128],
                                rhs=qT[pb:pb + 64, qc * 512:(qc + 1) * 512],
                                start=True, stop=True)
                        else:
                            w = 128 * rb + 128   # masked prefix width
                            lo = 384 - 128 * rb  # band-mask column offset
                            nc.tensor.matmul(
                                st[:, c0:c0 + 512],
                                lhsT=kT[pb:pb + 64, kb * 128:(kb + 1) * 128],
                                rhs=qT[pb:pb + 64, qc * 512:(qc + 1) * 512],
                                start=True, stop=False, skip_group_check=True)
                            # single K=128 mask matmul — two 64-row-tiled ones
                            # would run as concurrent row tiles on the same
                            # PSUM bank, which is fatal on HW
                            nc.tensor.matmul(
                                st[:, c0:c0 + w],
                                lhsT=negI_t, rhs=bm_t[:, lo:lo + w],
                                start=False, stop=True, skip_group_check=True)
                    ex = expp.tile([128, 512 * GS], F16, tag="expst",
                                   name=f"ex{h}_{qc}_{g}")
                    nc.scalar.activation(out=ex, in_=st, func=AF.Exp)
                    for kk in range(GS):
                        kb = g * GS + kk
                        rb = kb - 4 * qc
                        off = 128 * rb if rb > 0 else 0
                        nc.tensor.matmul(
                            ot[0:65, off:512],
                            lhsT=vt[kb][:, h, :],
                            rhs=ex[:, kk * 512 + off:kk * 512 + 512],
                            start=(kb == 0), stop=(kb == nk - 1),
                            skip_group_check=True)
                # drain + normalize this q-chunk:
                # outU[rows of head h] = ot[0:64] * (1 / rowsum)
                r0 = sub * 64
                rs = rsp.tile([128, 512], F32, tag="rs", name=f"rs{h}_{qc}")
                nc.vector.reciprocal(out=rs[64:65, :], in_=ot[64:65, :])
                # broadcast the reciprocal row across 64 partitions via a
                # DRAM bounce (DMA can replicate from a DRAM source)
                dr = drp.tile([1, 512], F32, tag="dr", name=f"dr{h}_{qc}")
                nc.sync.dma_start(out=dr, in_=rs[64:65, :])
                bc = bass.AP(tensor=dr.tensor, offset=dr.offset,
                             ap=[[0, 64]] + [list(d) for d in dr.ap])
                rseg = rsegp.tile([64, 512], F32, tag="rseg", name=f"rg{h}_{qc}")
                nc.sync.dma_start(out=rseg, in_=bc)
                nc.vector.tensor_mul(
                    outU[hp][r0:r0 + 64, qc * 512:(qc + 1) * 512],
                    ot[0:64, :], rseg)

        if phase == "attn":
            for j in range(4):
                nc.sync.dma_start(out=projT[j * 128:(j + 1) * 128, :], in_=outU[j])
            return

        # ---- partial projection: projT[o, t] = wpT.T @ outU ----
        es_attn.close()  # free attention temp space
        poutp = ctx.enter_context(tc.tile_pool(name="poutp", bufs=2))
        with tc.tile_pool(name="wpp", bufs=1) as wpp:
            wp = []
            for j in range(4):
                wp_t = wpp.tile([128, 1024], F16, name=f"wp{j}")
                nc.sync.dma_start(out=wp_t, in_=wpT[j * 128:(j + 1) * 128, :])
                wp.append(wp_t)
            for ob in range(8):
                po = poutp.tile([128, T], F32, tag="pout", name=f"po{ob}")
                for tcc in range(TC):
                    ps = psum.tile([128, 512], F32, tag="acc", bufs=2,
                                   name=f"pps{ob}_{tcc}")
                    for j in range(4):
                        nc.tensor.matmul(
                            ps, lhsT=wp[j][:, ob * 128:(ob + 1) * 128],
                            rhs=outU[j][:, tcc * 512:(tcc + 1) * 512],
                            start=(j == 0), stop=(j == 3))
                    nc.vector.tensor_copy(out=po[:, tcc * 512:(tcc + 1) * 512], in_=ps)
                nc.sync.dma_start(out=projT[ob * 128:(ob + 1) * 128, :], in_=po)


def build_nc(phase="all"):
    global _CACHED_NC
    if _CACHED_NC is not None and _CACHED_NC[0] == phase:
        return _CACHED_NC[1]
    nc = bacc.Bacc("TRN2", target_bir_lowering=False, debug=False,
                   num_devices=N_CORES)
    xT = nc.dram_tensor("xT", [D, T], F16, kind="ExternalInput").ap()
    wqkT = nc.dram_tensor("wqkT", [8, CB, 128, 128], F16, kind="ExternalInput").ap()
    wvT = nc.dram_tensor("wvT", [D, 512], F16, kind="ExternalInput").ap()
    wpT = nc.dram_tensor("wpT", [512, D], F16, kind="ExternalInput").ap()
    stepm = nc.dram_tensor("stepm", [128, 512], F16, kind="ExternalInput").ap()
    negI = nc.dram_tensor("negI", [128, 128], F16, kind="ExternalInput").ap()
    projT = nc.dram_tensor("projT", [D, T], F32, kind="ExternalOutput").ap()

    with tile.TileContext(nc) as t:
        _emit(t, xT, wqkT, wvT, wpT, stepm, negI, projT, phase=phase)
    nc.compile()
    _CACHED_NC = (phase, nc)
    return nc


def make_in_maps(x, W_qkv, W_proj):
    x = np.asarray(x, dtype=np.float32)
    W_qkv = np.asarray(W_qkv, dtype=np.float32)
    W_proj = np.asarray(W_proj, dtype=np.float32)

    # band mask: bm[k, c] = 1 iff c < k + 384  (see _emit diagonal masking)
    stepm = (np.arange(512)[None, :] <
             np.arange(128)[:, None] + 384).astype(np.float16)
    negI = (NEG * np.eye(128)).astype(np.float16)

    in_maps = []
    for core in range(N_CORES):
        b, half = core // 2, core % 2
        s = 512 * half
        # fold the 1/sqrt(HD) attention scale into the Q weights
        wq = W_qkv[s:s + 512] * np.float32(1.0 / np.sqrt(HD))
        wk = W_qkv[1024 + s:1024 + s + 512]
        wvv = W_qkv[2048 + s:2048 + s + 512]
        wcatT = np.ascontiguousarray(np.concatenate([wq, wk], axis=0).T)  # [c, f]
        wqkT = np.ascontiguousarray(
            wcatT.reshape(8, 128, 8, 128).transpose(2, 0, 1, 3))  # [fb, cb, c, f]
        in_maps.append({
            "xT": np.ascontiguousarray(x[b].T).astype(np.float16),
            "wqkT": wqkT.astype(np.float16),
            "wvT": np.ascontiguousarray(wvv.T).astype(np.float16),
            "wpT": np.ascontiguousarray(W_proj[:, s:s + 512].T).astype(np.float16),
            "stepm": stepm,
            "negI": negI,
        })
    return in_maps


def gather_output(results, b_proj):
    b_proj = np.asarray(b_proj, dtype=np.float32)
    out = np.empty((B, T, D), dtype=np.float32)
    for b in range(B):
        p = results[2 * b]["projT"] + results[2 * b + 1]["projT"]  # [D, T]
        out[b] = p.T + b_proj[None, :]
    return out


def run(x, W_qkv, W_proj, b_proj, trace=False, tmpdir=None):
    nc = build_nc()
    in_maps = make_in_maps(x, W_qkv, W_proj)
    if trace:
        bass_utils.upload_artifacts = lambda d: d
    res = bass_utils.run_bass_kernel_spmd(
        nc, in_maps, core_ids=list(range(N_CORES)), trace=trace, tmpdir=tmpdir)
    return gather_output(res.results, b_proj), res


def kernel(x, W_qkv, W_proj, b_proj):
    out, _ = run(x, W_qkv, W_proj, b_proj)
    return out


# revision 31
# speedup vs baseline: 3.8117x; 1.5662x over previous
"""Multi-head causal self-attention on 8 Trainium2 NeuronCores.

Sharding: core = (batch b, head-half). Each of the 8 cores computes
attention for 8 of the 16 heads of one of the 4 batch elements, plus the
partial output projection over its 512 feature columns. Host sums the two
partial projections per batch and adds the bias.

All device tensors are kept transposed (feature-major) so every matmul
contraction lands on the partition axis:
  QK^T:  S^T[k,q] = Kz^T_blk.T @ Q^T_chunk          (contraction 128)
  AV:    outT[d,q] = V_ext_blk.T @ expS^T_blk       (contraction k=128)
Per-head K^T tiles are zero-padded to the full 128 contraction rows so
every matmul runs in the same (128-row) PE mode — mixing 64-row-tiled and
full matmuls costs a PE drain per mode switch.

V carries an extra ones-column so row 64 of the AV accumulator is the
softmax row sum. Causal masking: S^T is computed full-width; after the
exp, only the mixed 128x128 block of each diagonal-band tile is zeroed
with a triangular fp16 multiply (the fully-masked prefix columns are
never read by the AV matmuls). The row-sum reciprocal is computed across
128 partitions (DMA repack [1,512] -> [128,4]) to dodge the DVE's
serial iterative-divide cost, broadcast via a DRAM bounce, and applied
in one fused tensor-tensor multiply that also casts the output to fp16.

Matmul operands are fp16 (fp32 streams at half rate on the TRN2 PE; fp16
products are exact in the fp32 accumulator, so the only loss is input
rounding, ~5e-4). Softmax statistics stay fp32. qkv compute for each
head-pair is emitted between attention blocks so the TensorE work
overlaps the exp-bound attention pipeline.
"""

import numpy as np

import concourse.bass as bass
import concourse.tile as tile
from concourse import bacc, mybir
from concourse import bass_utils

F32 = mybir.dt.float32
F16 = mybir.dt.float16
AF = mybir.ActivationFunctionType

B, T, D, H, HD = 4, 2048, 1024, 16, 64
N_CORES = 8
HL = 8          # heads per core (local)
CB = 8          # c (contraction) blocks of 128
TB = 16         # t blocks of 128
TC = 4          # t chunks of 512
GS = 2          # k-blocks per exp group (staging = [128, 512*GS] psum)

_CACHED_NC = None


def _emit(tc, xT, wqkT, wvT, wpT, mixm, projT):
    nc = tc.nc
    from contextlib import ExitStack

    with ExitStack() as ctx:
        consts = ctx.enter_context(tc.tile_pool(name="consts", bufs=1))
        psum = ctx.enter_context(tc.tile_pool(name="psum", bufs=1, space="PSUM"))
        vtp = ctx.enter_context(tc.tile_pool(name="vtp", bufs=1))
        qkp = ctx.enter_context(tc.tile_pool(name="qkp", bufs=1))
        xsp = ctx.enter_context(tc.tile_pool(name="xsp", bufs=1))
        wvp = ctx.enter_context(tc.tile_pool(name="wvp", bufs=1))
        wqsp = ctx.enter_context(tc.tile_pool(name="wqsp", bufs=16))
        outup = ctx.enter_context(tc.tile_pool(name="outup", bufs=1))
        expp = ctx.enter_context(tc.tile_pool(name="expp", bufs=3))
        tmpp = ctx.enter_context(tc.tile_pool(name="tmpp", bufs=4))
        rpkp = ctx.enter_context(tc.tile_pool(name="rpkp", bufs=4))
        rsegp = ctx.enter_context(tc.tile_pool(name="rsegp", bufs=4))
        drp = ctx.enter_context(tc.tile_pool(name="drp", bufs=4, space="DRAM"))
        poutp = ctx.enter_context(tc.tile_pool(name="poutp", bufs=2))
        wpp = ctx.enter_context(tc.tile_pool(name="wpp", bufs=1))

        # ---- consts ----
        mix_t = consts.tile([128, 128], F16, name="mix_t")
        nc.sync.dma_start(out=mix_t, in_=mixm)

        # ---- load x^T and V weights ----
        xs = []
        for cb in range(CB):
            x_t = xsp.tile([128, T], F16, name=f"xs{cb}")
            nc.sync.dma_start(out=x_t, in_=xT[cb * 128:(cb + 1) * 128, :])
            xs.append(x_t)
        wv = []
        for cb in range(CB):
            wv_t = wvp.tile([128, 512], F16, name=f"wv{cb}")
            nc.sync.dma_start(out=wv_t, in_=wvT[cb * 128:(cb + 1) * 128, :])
            wv.append(wv_t)

        # ---- V = x @ Wv^T, stored [128, 8 heads, 65] with ones col ----
        vt = []
        for tb in range(TB):
            ps = psum.tile([128, 512], F32, tag="acc", bufs=2, name=f"vps{tb}")
            for cb in range(CB):
                nc.tensor.matmul(
                    ps, lhsT=xs[cb][:, tb * 128:(tb + 1) * 128], rhs=wv[cb],
                    start=(cb == 0), stop=(cb == CB - 1))
            v_t = vtp.tile([128, HL, 65], F16, name=f"vt{tb}")
            nc.vector.memset(v_t[:, :, 64:65], 1.0)
            nc.vector.tensor_copy(
                out=v_t[:, :, 0:64],
                in_=ps.rearrange("p (h d) -> p h d", h=HL))
            vt.append(v_t)

        # Q^T tiles per head pair; zero-padded per-head K^T tiles
        qq = [None] * 4
        ktz = [None] * HL
        outU = [outup.tile([128, T], F16, name=f"outU{j}") for j in range(4)]

        def emit_qkt(hp):
            """qkv for head pair hp: Q^T tile + 2 zero-padded K^T tiles."""
            # Q features (fb = hp)
            wqs = []
            for cb in range(CB):
                w_t = wqsp.tile([128, 128], F16, tag="wqs", name=f"wq{hp}_{cb}")
                nc.sync.dma_start(out=w_t, in_=wqkT[hp, cb])
                wqs.append(w_t)
            q_t = qkp.tile([128, T], F16, name=f"qq{hp}")
            for tcc in range(TC):
                ps = psum.tile([128, 512], F32, tag="acc", bufs=2,
                               name=f"qps{hp}_{tcc}")
                for cb in range(CB):
                    nc.tensor.matmul(
                        ps, lhsT=wqs[cb], rhs=xs[cb][:, tcc * 512:(tcc + 1) * 512],
                        start=(cb == 0), stop=(cb == CB - 1))
                nc.vector.tensor_copy(out=q_t[:, tcc * 512:(tcc + 1) * 512], in_=ps)
            qq[hp] = q_t
            # K features (fb = 4 + hp), split per head with zero padding
            wks = []
            for cb in range(CB):
                w_t = wqsp.tile([128, 128], F16, tag="wqs", name=f"wk{hp}_{cb}")
                nc.sync.dma_start(out=w_t, in_=wqkT[4 + hp, cb])
                wks.append(w_t)
            kza = qkp.tile([128, T], F16, name=f"kz{2 * hp}")
            kzb = qkp.tile([128, T], F16, name=f"kz{2 * hp + 1}")
            nc.gpsimd.memset(kza[64:128, :], 0.0)
            nc.gpsimd.memset(kzb[0:64, :], 0.0)
            for tcc in range(TC):
                ps = psum.tile([128, 512], F32, tag="acc", bufs=2,
                               name=f"kps{hp}_{tcc}")
                for cb in range(CB):
                    nc.tensor.matmul(
                        ps, lhsT=wks[cb], rhs=xs[cb][:, tcc * 512:(tcc + 1) * 512],
                        start=(cb == 0), stop=(cb == CB - 1))
                nc.vector.tensor_copy(
                    out=kza[0:64, tcc * 512:(tcc + 1) * 512], in_=ps[0:64, :])
                nc.vector.tensor_copy(
                    out=kzb[64:128, tcc * 512:(tcc + 1) * 512], in_=ps[64:128, :])
            ktz[2 * hp] = kza
            ktz[2 * hp + 1] = kzb

        def emit_attention(h):
            hp, sub = h // 2, h % 2
            qT = qq[hp]
            kz = ktz[h]
            for qc in range(TC):
                nk = 4 * qc + 4
                ot = psum.tile([128, 512], F32, tag="ott", bufs=2,
                               name=f"ot{h}_{qc}")
                for g in range(nk // GS):
                    st = psum.tile([128, 512 * GS], F32, tag="stag", bufs=2,
                                   name=f"st{h}_{qc}_{g}")
                    for kk in range(GS):
                        kb = g * GS + kk
                        nc.tensor.matmul(
                            st[:, kk * 512:(kk + 1) * 512],
                            lhsT=kz[:, kb * 128:(kb + 1) * 128],
                            rhs=qT[:, qc * 512:(qc + 1) * 512],
                            start=True, stop=True)
                    ex = expp.tile([128, 512 * GS], F16, tag="expst",
                                   name=f"ex{h}_{qc}_{g}")
                    nc.scalar.activation(out=ex, in_=st, func=AF.Exp)
                    for kk in range(GS):
                        kb = g * GS + kk
                        rb = kb - 4 * qc
                        off = 128 * rb if rb > 0 else 0
                        if rb >= 0:
                            # zero the mixed causal block (cols off..off+128)
                            mixs = ex[:, kk * 512 + off:kk * 512 + off + 128]
                            nc.vector.tensor_mul(mixs, mixs, mix_t)
                        nc.tensor.matmul(
                            ot[0:65, off:512],
                            lhsT=vt[kb][:, h, :],
                            rhs=ex[:, kk * 512 + off:kk * 512 + 512],
                            start=(kb == 0), stop=(kb == nk - 1),
                            skip_group_check=True)
                # evict + normalize: outU[head rows] = ot[0:64] / rowsum
                r0 = sub * 64
                tmp = tmpp.tile([65, 512], F32, tag="tmp", name=f"tm{h}_{qc}")
                nc.vector.tensor_copy(out=tmp, in_=ot[0:65, :])
                # reciprocal across 128 partitions: repack [1,512] -> [128,4]
                rpk = rpkp.tile([128, 4], F32, tag="rpk", name=f"rp{h}_{qc}")
                nc.sync.dma_start(out=rpk, in_=tmp[64:65, :])
                nc.vector.reciprocal(out=rpk, in_=rpk)
                dr = drp.tile([1, 512], F32, tag="dr", name=f"dr{h}_{qc}")
                nc.sync.dma_start(out=dr, in_=rpk)
                bc = bass.AP(tensor=dr.tensor, offset=dr.offset,
                             ap=[[0, 64]] + [list(d) for d in dr.ap])
                rseg = rsegp.tile([64, 512], F32, tag="rseg", name=f"rg{h}_{qc}")
                nc.sync.dma_start(out=rseg, in_=bc)
                nc.vector.tensor_mul(
                    outU[hp][r0:r0 + 64, qc * 512:(qc + 1) * 512],
                    tmp[0:64, :], rseg)

        for hp in range(4):
            emit_qkt(hp)
            emit_attention(2 * hp)
            emit_attention(2 * hp + 1)

        # ---- partial projection: projT[o, t] = wpT.T @ outU ----
        wp = []
        for j in range(4):
            wp_t = wpp.tile([128, 1024], F16, name=f"wp{j}")
            nc.sync.dma_start(out=wp_t, in_=wpT[j * 128:(j + 1) * 128, :])
            wp.append(wp_t)
        for ob in range(8):
            po = poutp.tile([128, T], F32, tag="pout", name=f"po{ob}")
            for tcc in range(TC):
                ps = psum.tile([128, 512], F32, tag="acc", bufs=2,
                               name=f"pps{ob}_{tcc}")
                for j in range(4):
                    nc.tensor.matmul(
                        ps, lhsT=wp[j][:, ob * 128:(ob + 1) * 128],
                        rhs=outU[j][:, tcc * 512:(tcc + 1) * 512],
                        start=(j == 0), stop=(j == 3))
                nc.scalar.copy(out=po[:, tcc * 512:(tcc + 1) * 512], in_=ps)
            nc.sync.dma_start(out=projT[ob * 128:(ob + 1) * 128, :], in_=po)


def build_nc():
    global _CACHED_NC
    if _CACHED_NC is not None:
        return _CACHED_NC
    nc = bacc.Bacc("TRN2", target_bir_lowering=False, debug=False,
                   num_devices=N_CORES)
    xT = nc.dram_tensor("xT", [D, T], F16, kind="ExternalInput").ap()
    wqkT = nc.dram_tensor("wqkT", [8, CB, 128, 128], F16, kind="ExternalInput").ap()
    wvT = nc.dram_tensor("wvT", [D, 512], F16, kind="ExternalInput").ap()
    wpT = nc.dram_tensor("wpT", [512, D], F16, kind="ExternalInput").ap()
    mixm = nc.dram_tensor("mixm", [128, 128], F16, kind="ExternalInput").ap()
    projT = nc.dram_tensor("projT", [D, T], F32, kind="ExternalOutput").ap()

    with tile.TileContext(nc) as t:
        _emit(t, xT, wqkT, wvT, wpT, mixm, projT)
    nc.compile()
    _CACHED_NC = nc
    return nc


def make_in_maps(x, W_qkv, W_proj):
    x = np.asarray(x, dtype=np.float32)
    W_qkv = np.asarray(W_qkv, dtype=np.float32)
    W_proj = np.asarray(W_proj, dtype=np.float32)

    # mixed-block causal mask: keep (1.0) iff qq >= k
    mixm = (np.arange(128)[None, :] >=
            np.arange(128)[:, None]).astype(np.float16)

    in_maps = []
    for core in range(N_CORES):
        b, half = core // 2, core % 2
        s = 512 * half
        # fold the 1/sqrt(HD) attention scale into the Q weights
        wq = W_qkv[s:s + 512] * np.float32(1.0 / np.sqrt(HD))
        wk = W_qkv[1024 + s:1024 + s + 512]
        wvv = W_qkv[2048 + s:2048 + s + 512]
        wcatT = np.ascontiguousarray(np.concatenate([wq, wk], axis=0).T)  # [c, f]
        wqkT = np.ascontiguousarray(
            wcatT.reshape(8, 128, 8, 128).transpose(2, 0, 1, 3))  # [fb, cb, c, f]
        in_maps.append({
            "xT": np.ascontiguousarray(x[b].T).astype(np.float16),
            "wqkT": wqkT.astype(np.float16),
            "wvT": np.ascontiguousarray(wvv.T).astype(np.float16),
            "wpT": np.ascontiguousarray(W_proj[:, s:s + 512].T).astype(np.float16),
            "mixm": mixm,
        })
    return in_maps


def gather_output(results, b_proj):
    b_proj = np.asarray(b_proj, dtype=np.float32)
    out = np.empty((B, T, D), dtype=np.float32)
    for b in range(B):
        p = results[2 * b]["projT"] + results[2 * b + 1]["projT"]  # [D, T]
        out[b] = p.T + b_proj[None, :]
    return out


def run(x, W_qkv, W_proj, b_proj, trace=False, tmpdir=None):
    nc = build_nc()
    in_maps = make_in_maps(x, W_qkv, W_proj)
    if trace:
        bass_utils.upload_artifacts = lambda d: d
    res = bass_utils.run_bass_kernel_spmd(
        nc, in_maps, core_ids=list(range(N_CORES)), trace=trace, tmpdir=tmpdir)
    return gather_output(res.results, b_proj), res


def kernel(x, W_qkv, W_proj, b_proj):
    out, _ = run(x, W_qkv, W_proj, b_proj)
    return out
